# revision 35
# baseline (speedup 1.0000x reference)
"""Distributed Trainium2 Bass kernel for a causal single-head attention layer.

Problem shapes (hardcoded): N=4, S=T=2048, D=1024, f32 I/O.
  q = query @ Wq.T ; k = key @ Wk.T ; v = value @ Wv.T
  y = softmax(mask(q k^T / sqrt(D))) v

Sharding over 8 NeuronCores: core c -> (batch n = c//2, parity h = c%2).
Each core owns 8 interleaved 128-row query blocks (global block G = 2j+h,
j=0..7), which balances the causal (triangular) score workload between the
two cores of a batch.

v2: K/V projections are NOT duplicated per pair anymore. Core h computes
the K and V projections only for its t-half [h*1024, (h+1)*1024) and the
halves are exchanged through pair-wise AllGather collectives (DRAM bounce
buffers), hiding the exchange under the Q projection and score compute.
This cuts per-core tensor work from ~7.8G to ~5.6G MACs.

Device compute is fp16 (TensorEngine runs 16-bit at 4x the fp32 rate) with
f32 PSUM accumulation; the host pre-transposes/casts inputs so no on-device
transposes are needed:
  kT[e,t_half] = wkT-weights x xkTh          (own t-half only)
  v[t_half,e]  = xvTh.T x wvT                (own t-half only)
  qT[e,s] = (Wq/32)T-weights x qT-inputs
  ST[t,s] = kT.T @ qT per 128-wide t-tile g, only for g <= 2j+1 (causal skip)
  EST = exp(ST) * mask  (mask data taken from the real attn_mask input)
  y[s,e], sums[s] = EST.T @ [v | 1]  (ones-column gives softmax denominator)
  out = y * (1/sums)
"""

import numpy as np

from concourse import bass, mybir, tile, bacc
from concourse.bass_utils import run_bass_kernel_spmd

P = 128
N_BATCH = 4
S = 2048   # full query length
T = 2048   # key/value length
D = 1024   # model dim
SL = 1024  # per-core query rows
TH = 1024  # per-core t-half for K/V projection
JB = SL // P   # 8 local s-blocks per core
GT = T // P    # 16 t-tiles
GH = TH // P   # 8 t-tiles owned locally
DO = D // P    # 8 outer tiles of the contraction dim
EO = D // P    # 8 outer tiles of the e dim
N_CORES = 8
PAIRS = [[0, 1], [2, 3], [4, 5], [6, 7]]

_GRAPH_CACHE = {}


def _build_graph():
    if "nc" in _GRAPH_CACHE:
        return _GRAPH_CACHE["nc"]

    fp16 = mybir.dt.float16
    f32 = mybir.dt.float32

    nc = bacc.Bacc("TRN2", target_bir_lowering=False, debug=False,
                   num_devices=N_CORES)

    xqT_e = nc.dram_tensor("xqT", [D, SL], fp16, kind="ExternalInput")
    xkT_e = nc.dram_tensor("xkTh", [D, TH], fp16, kind="ExternalInput")
    xvT_e = nc.dram_tensor("xvTh", [D, TH], fp16, kind="ExternalInput")
    wqT_e = nc.dram_tensor("wqT", [D, D], fp16, kind="ExternalInput")
    wkT_e = nc.dram_tensor("wkT", [D, D], fp16, kind="ExternalInput")
    wvT_e = nc.dram_tensor("wvT", [D, D], fp16, kind="ExternalInput")
    mask_e = nc.dram_tensor("maskT", [GT, P, P], fp16, kind="ExternalInput")
    out_e = nc.dram_tensor("out", [SL, D], fp16, kind="ExternalOutput")

    # collective bounce buffers (pair-wise AllGather works HBM->HBM).
    # K is exchanged as two 1MB gathers pipelined behind the projection
    # chunks; V as one 2MB gather. The serial CC worker then finishes the
    # V exchange well before the attention-value phase needs it.
    bk1_e = nc.dram_tensor("bk1", [P, EO, 512], fp16, kind="Internal")
    bk2_e = nc.dram_tensor("bk2", [P, EO, 512], fp16, kind="Internal")
    bv_e = nc.dram_tensor("bv", [P, GH, D], fp16, kind="Internal")
    gk1_e = nc.dram_tensor("gk1", [2, P, EO, 512], fp16, kind="Internal")
    gk2_e = nc.dram_tensor("gk2", [2, P, EO, 512], fp16, kind="Internal")
    gv_e = nc.dram_tensor("gv", [2, P, GH, D], fp16, kind="Internal")
    # tiny warmup collective: pays the NRT collective-stack setup cost
    # before the real exchanges need it
    bw_e = nc.dram_tensor("bw", [P, 16], fp16, kind="Internal")
    gw_e = nc.dram_tensor("gw", [2, P, 16], fp16, kind="Internal")

    xq_r = xqT_e.ap().rearrange("(o p) s -> p o s", p=P)
    xk_r = xkT_e.ap().rearrange("(o p) t -> p o t", p=P)
    xv_r = xvT_e.ap().rearrange("(o p) t -> p o t", p=P)
    wq_r = wqT_e.ap().rearrange("(o p) e -> p o e", p=P)
    wk_r = wkT_e.ap().rearrange("(o p) e -> p o e", p=P)
    wv_r = wvT_e.ap().rearrange("(o p) e -> p o e", p=P)

    with tile.TileContext(nc) as tc:
        with tc.tile_pool(name="persist", bufs=1) as persist:
            qT = persist.tile([P, EO, SL], fp16)       # [e-part, e-outer, s]
            kT = persist.tile([P, EO, T], fp16)        # [e-part, e-outer, t]
            vA = persist.tile([P, GT, D + 1], fp16)    # [t-part, t-outer, e+1]
            maskT = persist.tile([P, GT, P], fp16)     # [t-part, g, s-local]
            recip = persist.tile([P, JB], f32)

            nc.vector.memset(vA[:, :, D:D + 1], 1.0)

            with (
                tc.tile_pool(name="weights", bufs=3) as wpool,
                tc.tile_pool(name="xin", bufs=3) as xpool,
                tc.tile_pool(name="stage", bufs=2) as stpool,
            ):
                # ---- K projection (own t-half): kTh[e,t] = wkT.T @ xkTh ----
                # First chunk in (o, m) order: accumulate all 8 e-tiles across
                # 8 PSUM banks so the first matmul only needs the o=0 slices.
                kst = stpool.tile([P, EO, TH], fp16, tag="st", name="kst")
                # all weight loads up-front on the gpsimd DMA queue: the
                # scalar/vector engines stay free for PSUM->SBUF copies
                # (which gate PSUM recycling), sync carries the x inputs.
                wk = wpool.tile([P, DO, D], fp16, tag="w", name="wk")
                wv = wpool.tile([P, DO, D], fp16, tag="w", name="wv")
                wq = wpool.tile([P, DO, D], fp16, tag="w", name="wq")
                with nc.named_scope("ccwarm"):
                    nc.gpsimd.collective_compute(
                        "AllGather", mybir.AluOpType.bypass,
                        replica_groups=PAIRS,
                        ins=[bw_e.ap()], outs=[gw_e.ap()],
                    )
                nc.gpsimd.dma_start(wk[:, 0, :], wk_r[:, 0, :])
                nc.gpsimd.dma_start(wk[:, 1:DO, :], wk_r[:, 1:DO, :])
                with tc.tile_pool(name="qpsum", bufs=8, space="PSUM") as qpsum:
                    with nc.named_scope("projK"):
                        xks = []
                        for sc in range(TH // 512):
                            xk = xpool.tile([P, DO, 512], fp16, tag="x",
                                            name=f"xk{sc}")
                            if sc == 0:
                                nc.sync.dma_start(xk[:, 0, 0:256],
                                                  xk_r[:, 0, 0:256])
                                nc.sync.dma_start(xk[:, 0, 256:512],
                                                  xk_r[:, 0, 256:512])
                                nc.sync.dma_start(xk[:, 1:DO // 2, :],
                                                  xk_r[:, 1:DO // 2, 0:512])
                                nc.sync.dma_start(xk[:, DO // 2:DO, :],
                                                  xk_r[:, DO // 2:DO, 0:512])
                            else:
                                nc.sync.dma_start(
                                    xk[:, 0:DO // 2, :],
                                    xk_r[:, 0:DO // 2, 512:1024])
                                nc.sync.dma_start(
                                    xk[:, DO // 2:DO, :],
                                    xk_r[:, DO // 2:DO, 512:1024])
                            xks.append(xk)
                        nc.sync.dma_start(wv[:], wv_r[:])
                        # chunk 0: o=0 warmup pass across 8 PSUM banks so
                        # the first matmuls only need the tiny o=0 slices,
                        # then m-major so copies/bounce stream per e-tile.
                        pss = [qpsum.tile([P, 512], f32, tag="qp",
                                          name=f"kp0_{m2}")
                               for m2 in range(EO)]
                        for m in range(EO):
                            nc.tensor.matmul(
                                pss[m][:, 0:256], wk[:, 0, m * P:(m + 1) * P],
                                xks[0][:, 0, 0:256],
                                start=True, stop=False, skip_group_check=True,
                            )
                        for m in range(EO):
                            nc.tensor.matmul(
                                pss[m][:, 256:512], wk[:, 0, m * P:(m + 1) * P],
                                xks[0][:, 0, 256:512],
                                start=False, stop=False, skip_group_check=True,
                            )
                            for o in range(1, DO):
                                nc.tensor.matmul(
                                    pss[m][:], wk[:, o, m * P:(m + 1) * P],
                                    xks[0][:, o, :],
                                    start=False, stop=(o == DO - 1),
                                    skip_group_check=True,
                                )
                            nc.vector.tensor_copy(kst[:, m, 0:512], pss[m][:])
                        with nc.named_scope("swapK1"):
                            nc.gpsimd.dma_start(bk1_e.ap(), kst[:, :, 0:512])
                            nc.gpsimd.collective_compute(
                                "AllGather", mybir.AluOpType.bypass,
                                replica_groups=PAIRS,
                                ins=[bk1_e.ap()], outs=[gk1_e.ap()],
                            )
                        nc.gpsimd.dma_start(wq[:], wq_r[:])
                        # chunk 1: (m, o) order -- psum slots recycle
                        for m in range(EO):
                            ps = qpsum.tile([P, 512], f32, tag="qp",
                                            name=f"kp1_{m}")
                            for o in range(DO):
                                nc.tensor.matmul(
                                    ps[:], wk[:, o, m * P:(m + 1) * P],
                                    xks[1][:, o, :],
                                    start=(o == 0), stop=(o == DO - 1),
                                )
                            nc.vector.tensor_copy(kst[:, m, 512:1024], ps[:])
                # second K chunk bounce + gather
                with nc.named_scope("swapK2"):
                    nc.gpsimd.dma_start(bk2_e.ap(), kst[:, :, 512:1024])
                    nc.gpsimd.collective_compute(
                        "AllGather", mybir.AluOpType.bypass,
                        replica_groups=PAIRS,
                        ins=[bk2_e.ap()], outs=[gk2_e.ap()],
                    )

                with tc.tile_pool(name="ppsum", bufs=4, space="PSUM") as ppsum:
                    # ---- V projection (own half): v[t,e] = xvTh.T @ wvT ----
                    with nc.named_scope("projV"):
                        vst = stpool.tile([P, GH, D], fp16, tag="st",
                                          name="vst")
                        xvs = []
                        for sc in range(TH // 512):
                            xv = xpool.tile([P, DO, 512], fp16, tag="x",
                                            name=f"xv{sc}")
                            nc.sync.dma_start(
                                xv[:], xv_r[:, :, 512 * sc:512 * (sc + 1)])
                            xvs.append(xv)
                        for m in range(GH):
                            ps0 = ppsum.tile([P, 512], f32, tag="pp")
                            ps1 = ppsum.tile([P, 512], f32, tag="pp")
                            for o in range(DO):
                                lhsT = xvs[m // 4][:, o, (m % 4) * P:
                                                   (m % 4 + 1) * P]
                                nc.tensor.matmul(ps0[:], lhsT, wv[:, o, 0:512],
                                                 start=(o == 0),
                                                 stop=(o == DO - 1))
                                nc.tensor.matmul(ps1[:], lhsT,
                                                 wv[:, o, 512:1024],
                                                 start=(o == 0),
                                                 stop=(o == DO - 1))
                            nc.scalar.copy(vst[:, m, 0:512], ps0[:])
                            nc.vector.tensor_copy(vst[:, m, 512:1024], ps1[:])
                            if m == GH // 2 - 1:
                                nc.gpsimd.dma_start(
                                    bv_e.ap()[:, 0:GH // 2, :],
                                    vst[:, 0:GH // 2, :])
                        nc.gpsimd.dma_start(bv_e.ap()[:, GH // 2:GH, :],
                                            vst[:, GH // 2:GH, :])
                    with nc.named_scope("swapV"):
                        nc.gpsimd.collective_compute(
                            "AllGather", mybir.AluOpType.bypass,
                            replica_groups=PAIRS,
                            ins=[bv_e.ap()], outs=[gv_e.ap()],
                        )

                    # ---- Q projection: qT[e,s] = wqT.T @ xqT ----
                    with nc.named_scope("projQ"):
                        for sc in range(SL // 512):
                            xq = xpool.tile([P, DO, 512], fp16, tag="x",
                                            name=f"xq{sc}")
                            nc.sync.dma_start(
                                xq[:], xq_r[:, :, 512 * sc:512 * (sc + 1)])
                            for m in range(EO):
                                ps = ppsum.tile([P, 512], f32, tag="pp")
                                for o in range(DO):
                                    nc.tensor.matmul(
                                        ps[:], wq[:, o, m * P:(m + 1) * P],
                                        xq[:, o, :],
                                        start=(o == 0), stop=(o == DO - 1),
                                    )
                                nc.vector.tensor_copy(
                                    qT[:, m, 512 * sc:512 * (sc + 1)], ps[:])
                        nc.sync.dma_start(
                            maskT[:], mask_e.ap().rearrange("g p s -> p g s"))

                    # ---- gather read-back: full kT and vA ----
                    with nc.named_scope("readback"):
                        for r in range(2):
                            nc.gpsimd.dma_start(
                                kT[:, :, r * TH:r * TH + 512], gk1_e.ap()[r])
                        for r in range(2):
                            nc.gpsimd.dma_start(
                                kT[:, :, r * TH + 512:(r + 1) * TH],
                                gk2_e.ap()[r])
                        # 4 x 1MB pieces in AV-consumption order: tiles
                        # 0-3 land first so the j-loop starts unstalled
                        for r in range(2):
                            for half in range(2):
                                g0 = r * GH + half * (GH // 2)
                                nc.gpsimd.dma_start(
                                    vA[:, g0:g0 + GH // 2, 0:D],
                                    gv_e.ap()[r][:, half * (GH // 2):
                                                 (half + 1) * (GH // 2), :])

            # ---- scores + exp + mask, per t-tile g ----
            with tc.tile_pool(name="estp", bufs=1) as estp:
                est = estp.tile([P, GT, SL], fp16)     # [t-part, t-outer, s]
                with (
                    nc.named_scope("scores"),
                    tc.tile_pool(name="spsum", bufs=3, space="PSUM") as spsum,
                ):
                    for g in range(GT):
                        j0 = g // 2
                        s0 = j0 * P
                        ncols = SL - s0
                        ps = spsum.tile([P, 1024], f32, tag="sp")
                        n_first = min(512, ncols)
                        for c in range(EO):
                            lhsT = kT[:, c, g * P:(g + 1) * P]
                            nc.tensor.matmul(
                                ps[:, 0:n_first], lhsT, qT[:, c, s0:s0 + n_first],
                                start=(c == 0), stop=(c == EO - 1),
                            )
                            if ncols > 512:
                                nc.tensor.matmul(
                                    ps[:, 512:ncols], lhsT, qT[:, c, s0 + 512:SL],
                                    start=(c == 0), stop=(c == EO - 1),
                                )
                        nc.scalar.activation(
                            est[:, g, s0:SL], ps[:, 0:ncols],
                            mybir.ActivationFunctionType.Exp,
                        )
                        nc.vector.tensor_mul(
                            out=est[:, g, s0:s0 + P],
                            in0=est[:, g, s0:s0 + P],
                            in1=maskT[:, g, :],
                        )

                # ---- attention-value + row sums + normalize, per block j ----
                with nc.named_scope("av"):
                    with (
                        tc.tile_pool(name="bpsum", bufs=2, space="PSUM") as bpsum,
                        tc.tile_pool(name="yout", bufs=3) as ypool,
                    ):
                        for j in range(JB):
                            gmax = 2 * j + 2
                            ps = bpsum.tile([P, D + 1], f32, tag="bp")
                            for g in range(gmax):
                                lhsT = est[:, g, j * P:(j + 1) * P]
                                st = (g == 0)
                                sp = (g == gmax - 1)
                                # sums column first: on the last g the
                                # reciprocal overlaps the trailing matmuls
                                nc.tensor.matmul(ps[:, 1024:1025], lhsT,
                                                 vA[:, g, 1024:1025],
                                                 start=st, stop=sp)
                                nc.tensor.matmul(ps[:, 0:512], lhsT,
                                                 vA[:, g, 0:512],
                                                 start=st, stop=sp)
                                nc.tensor.matmul(ps[:, 512:1024], lhsT,
                                                 vA[:, g, 512:1024],
                                                 start=st, stop=sp)
                            nc.vector.reciprocal(recip[:, j:j + 1],
                                                 ps[:, D:D + 1])
                            yt = ypool.tile([P, D], fp16, tag="y")
                            for q2 in range(2):
                                c0 = q2 * 512
                                nc.vector.tensor_scalar_mul(
                                    yt[:, c0:c0 + 512], ps[:, c0:c0 + 512],
                                    recip[:, j:j + 1])
                                eng = nc.sync if q2 == 0 else nc.scalar
                                eng.dma_start(
                                    out_e.ap()[j * P:(j + 1) * P, c0:c0 + 512],
                                    yt[:, c0:c0 + 512])

    nc.compile()
    _GRAPH_CACHE["nc"] = nc
    return nc


def _s_index(h):
    return np.concatenate([np.arange(P) + (2 * j + h) * P for j in range(JB)])


def _prepare_in_maps(query, key, value, attn_mask, Wq, Wk, Wv):
    query = np.asarray(query, np.float32)
    key = np.asarray(key, np.float32)
    value = np.asarray(value, np.float32)
    attn_mask = np.asarray(attn_mask)
    Wq = np.asarray(Wq, np.float32)
    Wk = np.asarray(Wk, np.float32)
    Wv = np.asarray(Wv, np.float32)

    scale = np.float32(1.0 / np.sqrt(np.float32(D)))
    wqT = np.ascontiguousarray((Wq * scale).T).astype(np.float16)  # [d, e]
    wkT = np.ascontiguousarray(Wk.T).astype(np.float16)
    wvT = np.ascontiguousarray(Wv.T).astype(np.float16)

    in_maps = []
    for c in range(N_CORES):
        n, h = c // 2, c % 2
        sidx = _s_index(h)
        xqT = np.ascontiguousarray(query[n][sidx].T).astype(np.float16)
        kTn = key[n].T   # [d, t]
        vTn = value[n].T
        xkTh = np.ascontiguousarray(
            kTn[:, h * TH:(h + 1) * TH]).astype(np.float16)
        xvTh = np.ascontiguousarray(
            vTn[:, h * TH:(h + 1) * TH]).astype(np.float16)
        maskT = np.empty((GT, P, P), np.float16)
        for g in range(GT):
            j0 = g // 2
            G0 = 2 * j0 + h
            blk = attn_mask[G0 * P:(G0 + 1) * P, g * P:(g + 1) * P]  # [s, t]
            maskT[g] = np.ascontiguousarray(blk.T).astype(np.float16)
        in_maps.append({
            "xqT": xqT, "xkTh": xkTh, "xvTh": xvTh,
            "wqT": wqT, "wkT": wkT, "wvT": wvT, "maskT": maskT,
        })
    return in_maps


def run(trace=False, **inputs):
    nc = _build_graph()
    in_maps = _prepare_in_maps(**inputs)
    res = run_bass_kernel_spmd(nc, in_maps, list(range(N_CORES)), trace=trace)
    out = np.empty((N_BATCH, S, D), np.float32)
    for c in range(N_CORES):
        n, h = c // 2, c % 2
        out[n][_s_index(h)] = res.results[c]["out"].astype(np.float32)
    return out, res


def kernel(**inputs):
    out, _ = run(trace=False, **inputs)
    return out


# revision 37
# speedup vs baseline: 1.0997x; 1.0997x over previous
"""Distributed Trainium2 Bass kernel for a causal single-head attention layer.

Problem shapes (hardcoded): N=4, S=T=2048, D=1024, f32 I/O.
  q = query @ Wq.T ; k = key @ Wk.T ; v = value @ Wv.T
  y = softmax(mask(q k^T / sqrt(D))) v

Sharding over 8 NeuronCores: core c -> (batch n = c//2, parity h = c%2).
Each core owns 8 interleaved 128-row query blocks (global block G = 2j+h,
j=0..7), which balances the causal (triangular) score workload between the
two cores of a batch.

v2: K/V projections are NOT duplicated per pair anymore. Core h computes
the K and V projections only for its t-half [h*1024, (h+1)*1024) and the
halves are exchanged through pair-wise AllGather collectives (DRAM bounce
buffers), hiding the exchange under the Q projection and score compute.
This cuts per-core tensor work from ~7.8G to ~5.6G MACs.

Device compute is fp16 (TensorEngine runs 16-bit at 4x the fp32 rate) with
f32 PSUM accumulation; the host pre-transposes/casts inputs so no on-device
transposes are needed:
  kT[e,t_half] = wkT-weights x xkTh          (own t-half only)
  v[t_half,e]  = xvTh.T x wvT                (own t-half only)
  qT[e,s] = (Wq/32)T-weights x qT-inputs
  ST[t,s] = kT.T @ qT per 128-wide t-tile g, only for g <= 2j+1 (causal skip)
  EST = exp(ST) * mask  (mask data taken from the real attn_mask input)
  y[s,e], sums[s] = EST.T @ [v | 1]  (ones-column gives softmax denominator)
  out = y * (1/sums)
"""

import numpy as np

from concourse import bass, mybir, tile, bacc
from concourse.bass_utils import run_bass_kernel_spmd

P = 128
N_BATCH = 4
S = 2048   # full query length
T = 2048   # key/value length
D = 1024   # model dim
SL = 1024  # per-core query rows
TH = 1024  # per-core t-half for K/V projection
JB = SL // P   # 8 local s-blocks per core
GT = T // P    # 16 t-tiles
GH = TH // P   # 8 t-tiles owned locally
DO = D // P    # 8 outer tiles of the contraction dim
EO = D // P    # 8 outer tiles of the e dim
N_CORES = 8
PAIRS = [[0, 1], [2, 3], [4, 5], [6, 7]]

_GRAPH_CACHE = {}


def _build_graph():
    if "nc" in _GRAPH_CACHE:
        return _GRAPH_CACHE["nc"]

    fp16 = mybir.dt.float16
    f32 = mybir.dt.float32

    nc = bacc.Bacc("TRN2", target_bir_lowering=False, debug=False,
                   num_devices=N_CORES)

    xqT_e = nc.dram_tensor("xqT", [D, SL], fp16, kind="ExternalInput")
    xkT_e = nc.dram_tensor("xkTh", [D, TH], fp16, kind="ExternalInput")
    xvT_e = nc.dram_tensor("xvTh", [D, TH], fp16, kind="ExternalInput")
    wqT_e = nc.dram_tensor("wqT", [D, D], fp16, kind="ExternalInput")
    wkT_e = nc.dram_tensor("wkT", [D, D], fp16, kind="ExternalInput")
    wvT_e = nc.dram_tensor("wvT", [D, D], fp16, kind="ExternalInput")
    mask_e = nc.dram_tensor("maskT", [GT, P, P], fp16, kind="ExternalInput")
    out_e = nc.dram_tensor("out", [SL, D], fp16, kind="ExternalOutput")

    # collective bounce buffers (pair-wise AllGather works HBM->HBM).
    # K is exchanged as two 1MB gathers pipelined behind the projection
    # chunks; V as one 2MB gather. The serial CC worker then finishes the
    # V exchange well before the attention-value phase needs it.
    bk1_e = nc.dram_tensor("bk1", [P, EO, 512], fp16, kind="Internal")
    bk2_e = nc.dram_tensor("bk2", [P, EO, 512], fp16, kind="Internal")
    bv_e = nc.dram_tensor("bv", [P, GH, D], fp16, kind="Internal")
    gk1_e = nc.dram_tensor("gk1", [2, P, EO, 512], fp16, kind="Internal")
    gk2_e = nc.dram_tensor("gk2", [2, P, EO, 512], fp16, kind="Internal")
    gv_e = nc.dram_tensor("gv", [2, P, GH, D], fp16, kind="Internal")
    # tiny warmup collective: pays the NRT collective-stack setup cost
    # before the real exchanges need it
    bw_e = nc.dram_tensor("bw", [P, 16], fp16, kind="Internal")
    gw_e = nc.dram_tensor("gw", [2, P, 16], fp16, kind="Internal")

    xq_r = xqT_e.ap().rearrange("(o p) s -> p o s", p=P)
    xk_r = xkT_e.ap().rearrange("(o p) t -> p o t", p=P)
    xv_r = xvT_e.ap().rearrange("(o p) t -> p o t", p=P)
    wq_r = wqT_e.ap().rearrange("(o p) e -> p o e", p=P)
    wk_r = wkT_e.ap().rearrange("(o p) e -> p o e", p=P)
    wv_r = wvT_e.ap().rearrange("(o p) e -> p o e", p=P)

    with tile.TileContext(nc) as tc:
        with tc.tile_pool(name="persist", bufs=1) as persist:
            qT = persist.tile([P, EO, SL], fp16)       # [e-part, e-outer, s]
            kT = persist.tile([P, EO, T], fp16)        # [e-part, e-outer, t]
            vA = persist.tile([P, GT, D + 1], fp16)    # [t-part, t-outer, e+1]
            maskT = persist.tile([P, GT, P], fp16)     # [t-part, g, s-local]
            recip = persist.tile([P, JB], f32)

            nc.vector.memset(vA[:, :, D:D + 1], 1.0)

            with (
                tc.tile_pool(name="weights", bufs=3) as wpool,
                tc.tile_pool(name="xin", bufs=3) as xpool,
                tc.tile_pool(name="stage", bufs=2) as stpool,
            ):
                # ---- K projection (own t-half): kTh[e,t] = wkT.T @ xkTh ----
                # First chunk in (o, m) order: accumulate all 8 e-tiles across
                # 8 PSUM banks so the first matmul only needs the o=0 slices.
                kst = stpool.tile([P, EO, TH], fp16, tag="st", name="kst")
                # all weight loads up-front on the gpsimd DMA queue: the
                # scalar/vector engines stay free for PSUM->SBUF copies
                # (which gate PSUM recycling), sync carries the x inputs.
                wk = wpool.tile([P, DO, D], fp16, tag="w", name="wk")
                wv = wpool.tile([P, DO, D], fp16, tag="w", name="wv")
                wq = wpool.tile([P, DO, D], fp16, tag="w", name="wq")
                with nc.named_scope("ccwarm"):
                    nc.gpsimd.collective_compute(
                        "AllGather", mybir.AluOpType.bypass,
                        replica_groups=PAIRS,
                        ins=[bw_e.ap()], outs=[gw_e.ap()],
                    )
                nc.gpsimd.dma_start(wk[:, 0, :], wk_r[:, 0, :])
                nc.gpsimd.dma_start(wk[:, 1:DO, :], wk_r[:, 1:DO, :])
                with tc.tile_pool(name="qpsum", bufs=8, space="PSUM") as qpsum:
                    with nc.named_scope("projK"):
                        xks = []
                        for sc in range(TH // 512):
                            xk = xpool.tile([P, DO, 512], fp16, tag="x",
                                            name=f"xk{sc}")
                            if sc == 0:
                                nc.sync.dma_start(xk[:, 0, 0:256],
                                                  xk_r[:, 0, 0:256])
                                nc.sync.dma_start(xk[:, 0, 256:512],
                                                  xk_r[:, 0, 256:512])
                                nc.scalar.dma_start(xk[:, 1:DO // 2, :],
                                                    xk_r[:, 1:DO // 2, 0:512])
                                nc.sync.dma_start(xk[:, DO // 2:DO, :],
                                                  xk_r[:, DO // 2:DO, 0:512])
                            else:
                                nc.sync.dma_start(
                                    xk[:, 0:DO // 2, :],
                                    xk_r[:, 0:DO // 2, 512:1024])
                                nc.sync.dma_start(
                                    xk[:, DO // 2:DO, :],
                                    xk_r[:, DO // 2:DO, 512:1024])
                            xks.append(xk)
                        nc.sync.dma_start(wv[:], wv_r[:])
                        # chunk 0: o=0 warmup pass across 8 PSUM banks so
                        # the first matmuls only need the tiny o=0 slices,
                        # then m-major so copies/bounce stream per e-tile.
                        pss = [qpsum.tile([P, 512], f32, tag="qp",
                                          name=f"kp0_{m2}")
                               for m2 in range(EO)]
                        for m in range(EO):
                            nc.tensor.matmul(
                                pss[m][:, 0:256], wk[:, 0, m * P:(m + 1) * P],
                                xks[0][:, 0, 0:256],
                                start=True, stop=False, skip_group_check=True,
                            )
                        for m in range(EO):
                            nc.tensor.matmul(
                                pss[m][:, 256:512], wk[:, 0, m * P:(m + 1) * P],
                                xks[0][:, 0, 256:512],
                                start=False, stop=False, skip_group_check=True,
                            )
                            for o in range(1, DO):
                                nc.tensor.matmul(
                                    pss[m][:], wk[:, o, m * P:(m + 1) * P],
                                    xks[0][:, o, :],
                                    start=False, stop=(o == DO - 1),
                                    skip_group_check=True,
                                )
                            nc.vector.tensor_copy(kst[:, m, 0:512], pss[m][:])
                        with nc.named_scope("swapK1"):
                            nc.gpsimd.dma_start(bk1_e.ap(), kst[:, :, 0:512])
                            nc.gpsimd.collective_compute(
                                "AllGather", mybir.AluOpType.bypass,
                                replica_groups=PAIRS,
                                ins=[bk1_e.ap()], outs=[gk1_e.ap()],
                            )
                        nc.gpsimd.dma_start(wq[:], wq_r[:])
                        # chunk 1: (m, o) order -- psum slots recycle
                        for m in range(EO):
                            ps = qpsum.tile([P, 512], f32, tag="qp",
                                            name=f"kp1_{m}")
                            for o in range(DO):
                                nc.tensor.matmul(
                                    ps[:], wk[:, o, m * P:(m + 1) * P],
                                    xks[1][:, o, :],
                                    start=(o == 0), stop=(o == DO - 1),
                                )
                            nc.vector.tensor_copy(kst[:, m, 512:1024], ps[:])
                # second K chunk bounce + gather
                with nc.named_scope("swapK2"):
                    nc.gpsimd.dma_start(bk2_e.ap(), kst[:, :, 512:1024])
                    nc.gpsimd.collective_compute(
                        "AllGather", mybir.AluOpType.bypass,
                        replica_groups=PAIRS,
                        ins=[bk2_e.ap()], outs=[gk2_e.ap()],
                    )

                with tc.tile_pool(name="ppsum", bufs=4, space="PSUM") as ppsum:
                    # ---- V projection (own half): v[t,e] = xvTh.T @ wvT ----
                    with nc.named_scope("projV"):
                        vst = stpool.tile([P, GH, D], fp16, tag="st",
                                          name="vst")
                        xvs = []
                        for sc in range(TH // 512):
                            xv = xpool.tile([P, DO, 512], fp16, tag="x",
                                            name=f"xv{sc}")
                            nc.sync.dma_start(
                                xv[:], xv_r[:, :, 512 * sc:512 * (sc + 1)])
                            xvs.append(xv)
                        for m in range(GH):
                            ps0 = ppsum.tile([P, 512], f32, tag="pp")
                            ps1 = ppsum.tile([P, 512], f32, tag="pp")
                            for o in range(DO):
                                lhsT = xvs[m // 4][:, o, (m % 4) * P:
                                                   (m % 4 + 1) * P]
                                nc.tensor.matmul(ps0[:], lhsT, wv[:, o, 0:512],
                                                 start=(o == 0),
                                                 stop=(o == DO - 1))
                                nc.tensor.matmul(ps1[:], lhsT,
                                                 wv[:, o, 512:1024],
                                                 start=(o == 0),
                                                 stop=(o == DO - 1))
                            nc.scalar.copy(vst[:, m, 0:512], ps0[:])
                            nc.vector.tensor_copy(vst[:, m, 512:1024], ps1[:])
                            if m == GH // 2 - 1:
                                nc.gpsimd.dma_start(
                                    bv_e.ap()[:, 0:GH // 2, :],
                                    vst[:, 0:GH // 2, :])
                        nc.gpsimd.dma_start(bv_e.ap()[:, GH // 2:GH, :],
                                            vst[:, GH // 2:GH, :])
                    with nc.named_scope("swapV"):
                        nc.gpsimd.collective_compute(
                            "AllGather", mybir.AluOpType.bypass,
                            replica_groups=PAIRS,
                            ins=[bv_e.ap()], outs=[gv_e.ap()],
                        )

                    # ---- Q projection: qT[e,s] = wqT.T @ xqT ----
                    with nc.named_scope("projQ"):
                        for sc in range(SL // 512):
                            xq = xpool.tile([P, DO, 512], fp16, tag="x",
                                            name=f"xq{sc}")
                            nc.sync.dma_start(
                                xq[:], xq_r[:, :, 512 * sc:512 * (sc + 1)])
                            for m in range(EO):
                                ps = ppsum.tile([P, 512], f32, tag="pp")
                                for o in range(DO):
                                    nc.tensor.matmul(
                                        ps[:], wq[:, o, m * P:(m + 1) * P],
                                        xq[:, o, :],
                                        start=(o == 0), stop=(o == DO - 1),
                                    )
                                nc.vector.tensor_copy(
                                    qT[:, m, 512 * sc:512 * (sc + 1)], ps[:])
                        nc.sync.dma_start(
                            maskT[:], mask_e.ap().rearrange("g p s -> p g s"))

                    # ---- gather read-back: full kT and vA ----
                    with nc.named_scope("readback"):
                        for r in range(2):
                            nc.gpsimd.dma_start(
                                kT[:, :, r * TH:r * TH + 512], gk1_e.ap()[r])
                        for r in range(2):
                            nc.gpsimd.dma_start(
                                kT[:, :, r * TH + 512:(r + 1) * TH],
                                gk2_e.ap()[r])
                        for r in range(2):
                            for half in range(2):
                                g0 = r * GH + half * (GH // 2)
                                nc.gpsimd.dma_start(
                                    vA[:, g0:g0 + GH // 2, 0:D],
                                    gv_e.ap()[r][:, half * (GH // 2):
                                                 (half + 1) * (GH // 2), :])

            # ---- scores + exp + mask, per t-tile g ----
            with tc.tile_pool(name="estp", bufs=1) as estp:
                est = estp.tile([P, GT, SL], fp16)     # [t-part, t-outer, s]
                with (
                    nc.named_scope("scores"),
                    tc.tile_pool(name="spsum", bufs=3, space="PSUM") as spsum,
                ):
                    for g in range(GT):
                        j0 = g // 2
                        s0 = j0 * P
                        ncols = SL - s0
                        ps = spsum.tile([P, 1024], f32, tag="sp")
                        n_first = min(512, ncols)
                        for c in range(EO):
                            lhsT = kT[:, c, g * P:(g + 1) * P]
                            nc.tensor.matmul(
                                ps[:, 0:n_first], lhsT, qT[:, c, s0:s0 + n_first],
                                start=(c == 0), stop=(c == EO - 1),
                            )
                            if ncols > 512:
                                nc.tensor.matmul(
                                    ps[:, 512:ncols], lhsT, qT[:, c, s0 + 512:SL],
                                    start=(c == 0), stop=(c == EO - 1),
                                )
                        nc.scalar.activation(
                            est[:, g, s0:SL], ps[:, 0:ncols],
                            mybir.ActivationFunctionType.Exp,
                        )
                        nc.vector.tensor_mul(
                            out=est[:, g, s0:s0 + P],
                            in0=est[:, g, s0:s0 + P],
                            in1=maskT[:, g, :],
                        )

                # ---- attention-value + row sums + normalize, per block j ----
                with nc.named_scope("av"):
                    with (
                        tc.tile_pool(name="bpsum", bufs=2, space="PSUM") as bpsum,
                        tc.tile_pool(name="yout", bufs=3) as ypool,
                    ):
                        for j in range(JB):
                            gmax = 2 * j + 2
                            ps = bpsum.tile([P, D + 1], f32, tag="bp")
                            for g in range(gmax):
                                lhsT = est[:, g, j * P:(j + 1) * P]
                                st = (g == 0)
                                sp = (g == gmax - 1)
                                # sums column first: on the last g the
                                # reciprocal overlaps the trailing matmuls
                                nc.tensor.matmul(ps[:, 1024:1025], lhsT,
                                                 vA[:, g, 1024:1025],
                                                 start=st, stop=sp)
                                nc.tensor.matmul(ps[:, 0:512], lhsT,
                                                 vA[:, g, 0:512],
                                                 start=st, stop=sp)
                                nc.tensor.matmul(ps[:, 512:1024], lhsT,
                                                 vA[:, g, 512:1024],
                                                 start=st, stop=sp)
                            nc.vector.reciprocal(recip[:, j:j + 1],
                                                 ps[:, D:D + 1])
                            yt = ypool.tile([P, D], fp16, tag="y")
                            for q2 in range(2):
                                c0 = q2 * 512
                                nc.vector.tensor_scalar_mul(
                                    yt[:, c0:c0 + 512], ps[:, c0:c0 + 512],
                                    recip[:, j:j + 1])
                                eng = nc.sync if q2 == 0 else nc.scalar
                                eng.dma_start(
                                    out_e.ap()[j * P:(j + 1) * P, c0:c0 + 512],
                                    yt[:, c0:c0 + 512])

    nc.compile()
    _GRAPH_CACHE["nc"] = nc
    return nc


def _s_index(h):
    return np.concatenate([np.arange(P) + (2 * j + h) * P for j in range(JB)])


def _prepare_in_maps(query, key, value, attn_mask, Wq, Wk, Wv):
    query = np.asarray(query, np.float32)
    key = np.asarray(key, np.float32)
    value = np.asarray(value, np.float32)
    attn_mask = np.asarray(attn_mask)
    Wq = np.asarray(Wq, np.float32)
    Wk = np.asarray(Wk, np.float32)
    Wv = np.asarray(Wv, np.float32)

    scale = np.float32(1.0 / np.sqrt(np.float32(D)))
    wqT = np.ascontiguousarray((Wq * scale).T).astype(np.float16)  # [d, e]
    wkT = np.ascontiguousarray(Wk.T).astype(np.float16)
    wvT = np.ascontiguousarray(Wv.T).astype(np.float16)

    in_maps = []
    for c in range(N_CORES):
        n, h = c // 2, c % 2
        sidx = _s_index(h)
        xqT = np.ascontiguousarray(query[n][sidx].T).astype(np.float16)
        kTn = key[n].T   # [d, t]
        vTn = value[n].T
        xkTh = np.ascontiguousarray(
            kTn[:, h * TH:(h + 1) * TH]).astype(np.float16)
        xvTh = np.ascontiguousarray(
            vTn[:, h * TH:(h + 1) * TH]).astype(np.float16)
        maskT = np.empty((GT, P, P), np.float16)
        for g in range(GT):
            j0 = g // 2
            G0 = 2 * j0 + h
            blk = attn_mask[G0 * P:(G0 + 1) * P, g * P:(g + 1) * P]  # [s, t]
            maskT[g] = np.ascontiguousarray(blk.T).astype(np.float16)
        in_maps.append({
            "xqT": xqT, "xkTh": xkTh, "xvTh": xvTh,
            "wqT": wqT, "wkT": wkT, "wvT": wvT, "maskT": maskT,
        })
    return in_maps


def run(trace=False, **inputs):
    nc = _build_graph()
    in_maps = _prepare_in_maps(**inputs)
    res = run_bass_kernel_spmd(nc, in_maps, list(range(N_CORES)), trace=trace)
    out = np.empty((N_BATCH, S, D), np.float32)
    for c in range(N_CORES):
        n, h = c // 2, c % 2
        out[n][_s_index(h)] = res.results[c]["out"].astype(np.float32)
    return out, res


def kernel(**inputs):
    out, _ = run(trace=False, **inputs)
    return out
